# revision 44
# baseline (speedup 1.0000x reference)
"""Trainium2 Bass kernel for nn_AdapativeStepLayer (adaptive-step memory network).

Pure data-parallel over batch (B=256 -> 32 examples/core x 8 cores); the
whole 8-step recurrence runs in exact fp32 via column-output PE matmuls.

 - Every per-example contraction (scores K@u, attended e@K, update [m;att]@W)
   is a chain of 128x128-stationary fp32 matmuls with 1-2 output columns, so
   the arithmetic exactly tracks the fp32 reference (rel err ~6e-5 after all
   8 steps of the chaotic attention recurrence, vs ~1e-1 for a 12-bit-mantissa
   pipeline) while costing only a few PE cycles per matmul.
 - K is DMA'd once per example as [128 k, (kc d)] tiles; K^T (needed to
   contract over d in the scores) is built on device by PE transpose through
   a fp32 identity into PSUM, then copied to SBUF by ACT/DVE alternately.
 - Scores fold the question in via a per-example sq = K^T q precomputed at
   group load; each step's chain is seeded with an identity-matmul of sq, so
   the per-step critical path never touches the DVE for u = q + m.
 - Softmax: global-shift exp(s - 110) on ACT; Z is broadcast to all 128
   partitions and all dc blocks in one sweep of ones[128,128]-stationary
   matmuls, so normalization is just DVE reciprocal + multiply (PSUM->SBUF),
   writing the normalized attended straight into the snapshot buffer that
   the update matmuls and the host replay both read.  new_mem = tanh(upd)
   also lands directly in the snapshot buffer in column layout; the steady
   state has no transposes and a 6-hop PE->ACT->PE->DVE->PE->ACT chain.
 - Schedule: groups of 2 examples, 4 K-slots, 3-step-staggered phases: in
   phase p group p runs steps 0-2, group p-1 steps 3-5, group p-2 steps 6-7,
   so three independent chains always interleave on the engines; the fourth
   slot DMAs group p+2's K and its K^T prep pieces hide inside phase p+1.
 - Per-step new_mem/attended snapshots accumulate in SBUF and are DMA'd out
   once per group; the reference's halting while-loop (a global any) is
   replayed exactly on the host, which is exact because inactive scan steps
   are identities in the reference.
"""
import sys
sys.path.insert(0, "/opt/trn_rl_repo")
import numpy as np

B, KS, DS = 256, 512, 512
NCORES = 8
BPC = B // NCORES            # 32 examples per core
G = 2                        # examples per group
BPCP = 32                    # no padding: 16 uniform groups of 2
NG = BPCP // G               # 16 groups
NCH = DS // 128              # 4 chunks of 128
NSTEP = 8
GCOL = NCH * G               # 8 columns per group ((dc, b) layout)
CSHIFT = 110.0
ONE_MINUS_EPS = 0.99
MAX_COMP = 8

_CACHE = {}


def _fix_waits(nc):
    from concourse import mybir
    ctr = 0
    for fn in nc.m.functions:
        for bb in fn.blocks:
            insts = bb.instructions
            out = []
            changed = False
            for inst in insts:
                si = inst.sync_info
                if si is not None and si.on_wait:
                    keep = 1
                    waits = list(si.on_wait)
                    if len(waits) > keep:
                        hoist = waits[: len(waits) - keep]
                        remain = waits[len(waits) - keep:]
                        for w in hoist:
                            ctr += 1
                            nop = mybir.InstNoOp(
                                name=f"waitfix-nop-{id(nc)}-{ctr}",
                                engine=inst.engine, ins=[], outs=[])
                            nop.sync_info = mybir.SyncInfo(on_wait=[w], on_update=[])
                            try:
                                nop.bass_nofuse = True
                            except Exception:
                                pass
                            out.append(nop)
                        inst.sync_info = mybir.SyncInfo(
                            on_wait=remain, on_update=list(si.on_update or []))
                        changed = True
                out.append(inst)
            if changed:
                bb.instructions = out
    return ctr


def _build():
    import concourse.bass as bass
    import concourse.tile as tile
    from concourse import mybir

    f32 = mybir.dt.float32
    f32r = mybir.dt.float32r
    nc = bass.Bass()

    k_ext = nc.declare_dram_parameter("Kt", [BPCP, 128, NCH * DS], f32, isOutput=False)
    identb_ext = nc.declare_dram_parameter("IdentB", [128, 128], f32, isOutput=False)
    qm_ext = nc.declare_dram_parameter("qm_cols", [NG, 128, 2 * GCOL], f32, isOutput=False)
    w_ext = nc.declare_dram_parameter("Wt", [128, 32 * 128], f32, isOutput=False)
    snap_ext = nc.declare_dram_parameter("snap", [NG, 128, 2 * NSTEP * GCOL], f32, isOutput=True)

    AF = mybir.ActivationFunctionType
    OP = mybir.AluOpType

    with tile.TileContext(nc) as tc:
        with tc.tile_pool(name="const", bufs=1) as cpool, \
             tc.tile_pool(name="kbig", bufs=1) as kpool, \
             tc.tile_pool(name="work", bufs=1) as wk, \
             tc.tile_pool(name="psum", bufs=1, space="PSUM") as pp:

            wt = cpool.tile([128, 32 * 128], f32, name="wt")
            nc.sync.dma_start(wt[:], w_ext[:])
            onesj = cpool.tile([128, 128], f32, name="onesj")
            nc.gpsimd.memset(onesj[:], 1.0)
            neg_c = cpool.tile([128, 1], f32, name="neg_c")
            nc.gpsimd.memset(neg_c[:], -CSHIFT)
            identb = cpool.tile([128, 128], f32, name="identb")
            nc.sync.dma_start(identb[:], identb_ext[:])

            # per-example tiles; stationary block helpers:
            #  K block (kc, dc): [128 k, 128 d] at kslab[b] col kc*512 + dc*128
            #  KT block (dc, kc): [128 d, 128 k] at ktslab[b] col dc*512 + kc*128
            def kblk(ctx, b, kc, dc):
                return ctx["kslab"][b][:, kc * DS + dc * 128:kc * DS + dc * 128 + 128]

            def ktblk(ctx, b, dc, kc):
                return ctx["ktslab"][b][:, dc * KS + kc * 128:dc * KS + kc * 128 + 128]

            def load_ctx(gi):
                slot = gi % 4
                base = gi * G
                kslab, ktslab = [], []
                for b in range(G):
                    kt = kpool.tile([128, NCH * DS], f32, name=f"k_{gi}_{b}",
                                    tag=f"k_{slot}_{b}")
                    nc.sync.dma_start(kt[:], k_ext[base + b])
                    kslab.append(kt)
                    ktt = kpool.tile([128, NCH * KS], f32, name=f"kt_{gi}_{b}",
                                     tag=f"kt_{slot}_{b}")
                    ktslab.append(ktt)
                qm = wk.tile([128, 2 * GCOL], f32, name=f"qm_{gi}", tag=f"qm_{slot}")
                nc.sync.dma_start(qm[:], qm_ext[gi])
                qc = qm[:, 0:GCOL]
                m0 = qm[:, GCOL:2 * GCOL]
                snap = wk.tile([128, 2 * NSTEP * GCOL], f32, name=f"sn_{gi}",
                               tag=f"sn_{slot}")
                snapm = snap[:, 0:NSTEP * GCOL]
                snapa = snap[:, NSTEP * GCOL:2 * NSTEP * GCOL]

                sq = wk.tile([128, GCOL], f32, name=f"sq_{gi}", tag=f"sq_{slot}")
                sq_ps = pp.tile([128, 512], f32, name=f"sqp_{gi}", tag="ubank", bufs=3)
                return dict(gi=gi, slot=slot, base=base, kslab=kslab, ktslab=ktslab,
                            qc=qc, m0=m0, snap=snap, snapm=snapm, snapa=snapa,
                            sq=sq, sq_ps=sq_ps)

            def prep_pieces(ctx):
                """K^T build (PE transpose + ACT/DVE copy) and sq = K^T q,
                as a list of closures to intersperse between steps."""
                gi = ctx["gi"]
                pieces = []

                def transpose_piece(b, dh):
                    def go():
                        ktp = pp.tile([128, 2 * KS], f32, name=f"ktp_{gi}_{b}_{dh}",
                                      tag="ktps", bufs=1)
                        for di in range(2):
                            dc = dh * 2 + di
                            for kc in range(NCH):
                                c0 = kc * DS + dc * 128
                                nc.tensor.transpose(
                                    ktp[:, di * KS + kc * 128:di * KS + (kc + 1) * 128],
                                    ctx["kslab"][b][:, c0:c0 + 128], identb[:])
                        dst = ctx["ktslab"][b][:, dh * 2 * KS:(dh * 2 + 2) * KS]
                        if (b + dh) % 2 == 0:
                            nc.scalar.copy(dst, ktp[:])
                        else:
                            nc.vector.tensor_copy(dst, ktp[:])
                    return go

                def sq_piece(b):
                    def go():
                        sq_ps = ctx["sq_ps"]
                        for kc in range(NCH):
                            for dc in range(NCH):
                                nc.tensor.matmul(
                                    sq_ps[:, kc * G + b:kc * G + b + 1],
                                    ktblk(ctx, b, dc, kc),
                                    ctx["qc"][:, dc * G + b:dc * G + b + 1],
                                    start=(dc == 0), stop=(dc == NCH - 1))
                    return go

                def sq_copy():
                    nc.scalar.copy(ctx["sq"][:], ctx["sq_ps"][:, 0:GCOL])

                for b in range(G):
                    for dh in range(NCH // 2):
                        pieces.append(transpose_piece(b, dh))
                for b in range(G):
                    pieces.append(sq_piece(b))
                pieces.append(sq_copy)
                return pieces

            def emit_step(ctx, t):
                gi, slot = ctx["gi"], ctx["slot"]
                sfx = f"_{slot}"
                m_prev = (ctx["m0"][:, 0:GCOL] if t == 0
                          else ctx["snapm"][:, (t - 1) * GCOL:t * GCOL])

                # one PSUM bank per (group, step): all chains open/close
                # sequentially on the in-order PE, so they can share a
                # 2KB zero region.
                bank = pp.tile([128, 512], f32, name=f"bk_{gi}_{t}", tag="bank", bufs=3)
                s_ps = bank[:, 0:GCOL]
                a_ps = bank[:, GCOL:2 * GCOL]
                z_ps = bank[:, 2 * GCOL:3 * GCOL]
                ubank = pp.tile([128, 512], f32, name=f"ub_{gi}_{t}", tag="ubank", bufs=3)
                upd_ps = ubank[:, 0:GCOL]
                for b in range(G):
                    for kc in range(NCH):
                        col = s_ps[:, kc * G + b:kc * G + b + 1]
                        nc.tensor.matmul(col, identb[:],
                                         ctx["sq"][:, kc * G + b:kc * G + b + 1],
                                         start=True, stop=False)
                        for dc in range(NCH):
                            nc.tensor.matmul(
                                col, ktblk(ctx, b, dc, kc),
                                m_prev[:, dc * G + b:dc * G + b + 1],
                                start=False, stop=(dc == NCH - 1))

                # e = exp(s - 110)
                e = wk.tile([128, GCOL], f32, name=f"e_{gi}_{t}", tag="e" + sfx, bufs=3)
                nc.scalar.activation(e[:], s_ps, AF.Exp, bias=neg_c[:], scale=1.0)

                # Z broadcast to every partition and every dc block in one go:
                # ones[128,128]^T @ e accumulated over kc -> z[p, dc*G+b] = Z_b
                for dc in range(NCH):
                    for kc in range(NCH):
                        nc.tensor.matmul(z_ps[:, dc * G:(dc + 1) * G], onesj[:],
                                         e[:, kc * G:(kc + 1) * G],
                                         start=(kc == 0), stop=(kc == NCH - 1))

                # attended (unnormalized): a[:, dc*G+b] = sum_kc K(kc,dc)^T @ e[:, kc*G+b]
                for b in range(G):
                    for dc in range(NCH):
                        for kc in range(NCH):
                            nc.tensor.matmul(
                                a_ps[:, dc * G + b:dc * G + b + 1],
                                kblk(ctx, b, kc, dc),
                                e[:, kc * G + b:kc * G + b + 1],
                                start=(kc == 0), stop=(kc == NCH - 1))

                # 1/Z on all partitions (PSUM -> SBUF), then normalize att
                zinv_sb = wk.tile([128, GCOL], f32, name=f"zi_{gi}_{t}",
                                  tag="zinv" + sfx, bufs=3)
                nc.vector.reciprocal(zinv_sb[:], z_ps)
                att = ctx["snapa"][:, t * GCOL:(t + 1) * GCOL]
                nc.vector.tensor_tensor(att, a_ps, zinv_sb[:], OP.mult)

                # update: upd[:, oc*G..] = sum_ic W(ic,oc)^T @ x_ic ; x = [m; att]
                for oc in range(NCH):
                    for ic in range(8):
                        if ic < NCH:
                            xs = m_prev[:, ic * G:(ic + 1) * G]
                        else:
                            xs = att[:, (ic - NCH) * G:(ic - NCH + 1) * G]
                        nc.tensor.matmul(
                            upd_ps[:, oc * G:(oc + 1) * G],
                            wt[:, (ic * NCH + oc) * 128:(ic * NCH + oc + 1) * 128],
                            xs,
                            start=(ic == 0), stop=(ic == 7))
                nc.scalar.activation(ctx["snapm"][:, t * GCOL:(t + 1) * GCOL],
                                     upd_ps, AF.Tanh)

            def finish_ctx(ctx):
                nc.sync.dma_start(snap_ext[ctx["gi"]], ctx["snap"][:])

            def interleave(steps, pieces):
                """Emit step-closures with prep pieces distributed between."""
                n = len(steps)
                per = [len(pieces) * (i + 1) // n for i in range(n)]
                lo = 0
                for i, st in enumerate(steps):
                    st()
                    for p in pieces[lo:per[i]]:
                        p()
                    lo = per[i]

            # 3-slot schedule: two groups always computing (4-step
            # stagger), third slot loading + K^T prep hidden in the back
            # half of each interleave phase.
            def interleave_back(seq, pieces):
                n = len(seq)
                per = [len(pieces) * (i + 1) // n for i in range(n)]
                lo = 0
                for i, st in enumerate(seq):
                    st()
                    for p in pieces[lo:per[i]]:
                        p()
                    lo = per[i]

            # 3-step-staggered phases: in phase p, group p runs steps 0-2,
            # group p-1 runs 3-5, group p-2 runs 6-7 (then retires).  The
            # fourth slot loads group p+2 and its K^T prep pieces hide
            # inside the phase.
            # 3-step-staggered phases: in phase p, group p runs steps 0-2,
            # group p-1 runs 3-5, group p-2 runs 6-7 (then retires).  The
            # fourth slot loads group p+2; its K^T prep pieces are
            # interleaved into the NEXT phase, after the sub-DMAs land.
            ctxs = {}
            ctxs[0] = load_ctx(0)
            ctxs[1] = load_ctx(1)
            pend = {0: prep_pieces(ctxs[1])}
            for p in prep_pieces(ctxs[0]):
                p()
            for p in range(NG + 2):
                parts = []
                if p < NG:
                    parts.append((ctxs[p], [0, 1, 2]))
                if 0 <= p - 1 < NG:
                    parts.append((ctxs[p - 1], [3, 4, 5]))
                if 0 <= p - 2 < NG:
                    parts.append((ctxs[p - 2], [6, 7]))
                if p + 2 < NG:
                    ctxs[p + 2] = load_ctx(p + 2)
                    pend[p + 1] = prep_pieces(ctxs[p + 2])
                seq = []
                for r in range(3):
                    for c, steps in parts:
                        if r < len(steps):
                            seq.append(lambda cc=c, tt=steps[r]: emit_step(cc, tt))
                pieces = pend.pop(p, [])
                n = len(seq)
                h = min(3, max(0, n - 1))
                last = max(h, n - 2)
                per = [0 if i < h else len(pieces) * (i - h + 1) // max(1, last - h + 1)
                       for i in range(n)]
                for i in range(last, n):
                    per[i] = len(pieces)
                lo = 0
                for i, st in enumerate(seq):
                    st()
                    for pc in pieces[lo:per[i]]:
                        pc()
                    lo = per[i]
                if 0 <= p - 2 < NG:
                    finish_ctx(ctxs[p - 2])

    _fix_waits(nc)
    return nc


def _get_runner():
    if "nc" not in _CACHE:
        _CACHE["nc"] = _build()
    return _CACHE["nc"]


def _identb():
    return np.eye(128, dtype=np.float32)


def kernel(encoded_question, current_memory, encoded_knowledge, halting_weight, W_update):
    q = np.ascontiguousarray(np.asarray(encoded_question, np.float32))
    m0 = np.ascontiguousarray(np.asarray(current_memory, np.float32))
    Kf = np.ascontiguousarray(np.asarray(encoded_knowledge, np.float32))
    hw = np.asarray(halting_weight, np.float32)
    W = np.ascontiguousarray(np.asarray(W_update, np.float32))

    nc = _get_runner()

    def cols_layout(x):  # [G, 512] -> [128, (dc, b)]
        return np.ascontiguousarray(
            x.reshape(G, NCH, 128).transpose(2, 1, 0).reshape(128, GCOL))

    # W [1024, 512] -> [128, (ic, oc, 128)]
    Wt = np.ascontiguousarray(
        W.reshape(8, 128, NCH, 128).transpose(1, 0, 2, 3).reshape(128, 32 * 128))

    in_maps = []
    for c in range(NCORES):
        sl = slice(c * BPC, (c + 1) * BPC)
        qs, ms, Ks = q[sl], m0[sl], Kf[sl]
        Kt = np.ascontiguousarray(
            Ks.reshape(BPCP, NCH, 128, DS).transpose(0, 2, 1, 3).reshape(BPCP, 128, NCH * DS))
        qmc = np.concatenate(
            [np.stack([cols_layout(qs[i * G:(i + 1) * G]) for i in range(NG)]),
             np.stack([cols_layout(ms[i * G:(i + 1) * G]) for i in range(NG)])], axis=2)
        in_maps.append({
            "Kt": Kt,
            "IdentB": _identb(),
            "qm_cols": np.ascontiguousarray(qmc),
            "Wt": Wt,
        })

    from concourse.bass_utils import run_bass_kernel_spmd
    r = run_bass_kernel_spmd(nc, in_maps, core_ids=list(range(NCORES)))
    results = r.results

    # ---- unpack snapshots: [NG, 128, (t, dc, b)] -> [NSTEP, BPC, DS] ----
    new_mem_all = np.zeros((NSTEP, B, DS), np.float32)
    att_all = np.zeros((NSTEP, B, DS), np.float32)
    for c in range(NCORES):
        snap = results[c]["snap"]                    # [NG, 128, 2*NSTEP*GCOL]
        for half, dst in ((0, new_mem_all), (1, att_all)):
            sm = snap[:, :, half * NSTEP * GCOL:(half + 1) * NSTEP * GCOL]
            v = sm.reshape(NG, 128, NSTEP, NCH, G).transpose(2, 0, 4, 3, 1)
            dst[:, c * BPC:(c + 1) * BPC] = v.reshape(NSTEP, BPCP, DS)[:, :BPC]

    # ---- host-side exact replay of halting logic from snapshots ----
    p_all = 1.0 / (1.0 + np.exp(-(new_mem_all.astype(np.float64) @ hw.astype(np.float64))[:, :, 0]))

    mask = np.ones(B, bool)
    acc = np.zeros(B, np.float32)
    acc_cmp = np.zeros(B, np.float32)
    hop = np.zeros(B, np.float32)
    mem_acc = np.zeros((B, DS), np.float32)
    att_out = np.zeros((B, DS), np.float32)
    for t in range(NSTEP):
        active = bool(np.any((acc_cmp < ONE_MINUS_EPS) & (hop < MAX_COMP)))
        p = p_all[t].astype(np.float32)
        new_mask = (acc + p < ONE_MINUS_EPS) & mask
        nf = new_mask.astype(np.float32)
        hop_n = hop + nf
        cond = bool(np.any(new_mask & (hop_n < MAX_COMP)))
        if active:
            upd = np.where(cond, p * nf, 1.0 - p)[:, None].astype(np.float32)
            mem_acc = (new_mem_all[t] * upd + mem_acc).astype(np.float32)
            acc = (acc + p * nf).astype(np.float32)
            acc_cmp = (acc_cmp + p * mask.astype(np.float32)).astype(np.float32)
            mask, hop = new_mask, hop_n
            att_out = att_all[t]
    return mem_acc, att_out
